# revision 6
# baseline (speedup 1.0000x reference)
"""Trainium2 Bass kernel for the GRU decoder (nn_Decoder_13168369730058).

Math (from the reference):
  h0 = encoder_outputs[0, :, -1, :]                       # (128, 512)
  step 1:   h1 = gru_cell(x=0, h0)
  step t>1: h_t = gru_cell(h_{t-1}, h_{t-1})   (carry is (h_new, h_new))

Because x == h from step 2 on, the two GRU matmuls fuse into one:
  g  = h @ Wc.T          Wc = [Wih_r+Whh_r; Wih_z+Whh_z; Whh_n; Wih_n]  (2048, 512)
  r  = sigmoid(g_r + b_r)        b_r = b_ih_r + b_hh_r
  z  = sigmoid(g_z + b_z)
  n  = tanh(g_in + b_in + r * (g_hn + b_hn))     b_in = b_ih_n, b_hn = b_hh_n
  h' = (1 - z) * n + z * h
Step 1 is the same recurrence with Wc -> W_hh and no in-matmul (g_in = 0).

Distribution: data-parallel over batch, 16 rows per core on 8 cores, weights
replicated; the out_len recurrence is local to each core.

On-chip layout is fully transposed (H on partitions, batch on free dim): the
matmul runs with the weight tile as the stationary operand (lhsT = 128x128
fp16 block, FWL) and the transposed hidden state h^T (128, 16) as the moving
operand, producing g^T directly in PSUM.

Step latency = 64 MM-dispatches (~26.6 ns each, dispatch-floor bound) plus
the serial sigmoid/tanh chain, so the chain is squeezed hard:
  - gates issue in r, z, hn, in order; the joint sigmoid(r|z) fires as soon
    as the z accumulation lands, in the shadow of the hn/in matmuls;
  - DVE program order omz, rhn, pre_n, zh keeps the critical ops at the
    queue head the moment their matmul inputs land;
  - the final blend (1-z)*n + z*h is ONE tensor_tensor_scan over
    interleaved operands: out[2j] = n[j], out[2j+1] = omz[j]*n[j] + zh[j],
    with tanh/omz/zh writing directly into the interleaved scan operands;
  - state is fp16 only (no fp32 copy); the output DMA ships the fp16 scan
    tile and the host widens/deinterleaves to fp32;
  - all three gate biases are injected into PSUM by tiny rank-k matmuls so
    there are no DVE bias adds.
"""

import os
import numpy as np

import concourse.bacc as bacc
import concourse.mybir as mybir
import concourse.tile as tile
from concourse.bass_utils import run_bass_kernel_spmd

H = 512
BATCH = 128
N_CORES = int(os.environ.get("GRU_N_CORES", "8"))
T_STEPS = int(os.environ.get("GRU_T_STEPS", "1024"))
B_LOC = BATCH // N_CORES  # local batch per core (16)
SCALE_HOST = 1024.0
KT = H // 128             # 4 k-tiles

F32 = mybir.dt.float32
F16 = mybir.dt.float16
F8 = mybir.dt.float8e4
SCALE = 1024.0  # fp8/fp16 weights & biases stored x1024; descaled in ACT


def _build(T: int, b: int):
    """Build the Bass program: T steps, b batch rows per core."""
    nc = bacc.Bacc()

    w8_d = nc.dram_tensor("w8", [128, 96 * 128], F8, kind="ExternalInput")
    w16_d = nc.dram_tensor("w16", [128, 16 * 128], F16, kind="ExternalInput")
    # bias stationaries: row k of section s = bias[128k:128(k+1)] for that gate
    bst_d = nc.dram_tensor("bst", [16, 128], F16, kind="ExternalInput")
    ones8_d = nc.dram_tensor("ones8", [8, 8 * b], F16, kind="ExternalInput")
    ones4_d = nc.dram_tensor("ones4", [4, 4 * b], F16, kind="ExternalInput")
    h0_d = nc.dram_tensor("h0t", [128, 8 * b], F16, kind="ExternalInput")
    out_d = nc.dram_tensor("outT", [T, 128, 8 * b], F16, kind="ExternalOutput")

    sig = mybir.ActivationFunctionType.Sigmoid
    tanh = mybir.ActivationFunctionType.Tanh

    with tile.TileContext(nc) as tc:
        with (
            tc.tile_pool(name="singles", bufs=1) as singles,
            tc.tile_pool(name="state", bufs=2) as state,
            tc.tile_pool(name="work", bufs=2) as work,
            tc.tile_pool(name="psum", bufs=2, space="PSUM") as psum,
        ):
            w8_sb = singles.tile([128, 96 * 128], F8)
            nc.sync.dma_start(w8_sb[:], w8_d[:])
            w16_sb = singles.tile([128, 16 * 128], F16)
            nc.sync.dma_start(w16_sb[:], w16_d[:])
            brz_sb = singles.tile([8, 128], F16)
            nc.sync.dma_start(brz_sb[:], bst_d[0:8])
            bhn_sb = singles.tile([4, 128], F16)
            nc.sync.dma_start(bhn_sb[:], bst_d[8:12])
            bin_sb = singles.tile([4, 128], F16)
            nc.sync.dma_start(bin_sb[:], bst_d[12:16])
            ones8_sb = singles.tile([8, 8 * b], F16)
            nc.sync.dma_start(ones8_sb[:], ones8_d[:])
            ones4_sb = singles.tile([4, 4 * b], F16)
            nc.sync.dma_start(ones4_sb[:], ones4_d[:])

            # interleaved scan operands: sd0 = [0 | omz], sd1 = [n | zh].
            # sd0's even lanes must stay zero forever -> memset once here.
            sd0 = singles.tile([128, 8 * b], F16)
            nc.vector.memset(sd0[:], 0.0)
            sd1 = singles.tile([128, 8 * b], F16)
            nc.vector.memset(sd1[:], 0.0)

            ho = state.tile([128, 8 * b], F16, tag="ho")
            nc.sync.dma_start(ho[:], h0_d[:])

            # Warm-up: hardware allows ONE embedded sync wait per instruction;
            # have each engine observe the init DMA queues here so loop
            # instructions carry a single cross-engine wait.
            warm_ps = psum.tile([128, 8], F32, tag="warm", bufs=1)
            nc.tensor.matmul(warm_ps[:, 0:8], w8_sb[:, 0:128], ho[:, 1:16:2],
                             start=True, stop=True)
            nc.tensor.matmul(warm_ps[:, 0:8], w16_sb[:, 0:128], w16_sb[:, 0:8],
                             start=True, stop=True)
            nc.tensor.matmul(warm_ps[:, 0:1], brz_sb[:, 0:128], ones8_sb[:, 0:1],
                             start=True, stop=True)
            nc.tensor.matmul(warm_ps[:, 0:1], bhn_sb[:, 0:128], ones4_sb[:, 0:1],
                             start=True, stop=True)
            nc.tensor.matmul(warm_ps[:, 0:1], bin_sb[:, 0:128], ones4_sb[:, 0:1],
                             start=True, stop=True)

            for t in range(T):
                first = t == 0
                w8_base = (48 * 128) if first else 0

                rz_ps = psum.tile([128, 8 * b], F32, tag="rz")
                hn_ps = psum.tile([128, 4 * b], F32, tag="hn")
                in_ps = psum.tile([128, 4 * b], F32, tag="in")

                # bias seeds (start=True writes bias, sets has_written)
                nc.tensor.matmul(rz_ps[:], brz_sb[:], ones8_sb[:],
                                 start=True, stop=False, skip_group_check=True)
                nc.tensor.matmul(hn_ps[:], bhn_sb[:], ones4_sb[:],
                                 start=True, stop=False, skip_group_check=True)
                nc.tensor.matmul(in_ps[:], bin_sb[:], ones4_sb[:],
                                 start=True, stop=first, skip_group_check=True)

                def mm_block(w_sb, base, ps, ps_off, blk0, ntiles):
                    for tt in range(ntiles):
                        for k in range(KT):
                            blk = base + (blk0 + tt * KT + k) * 128
                            nc.tensor.matmul(
                                ps[:, (ps_off + tt) * b : (ps_off + tt + 1) * b],
                                w_sb[:, blk : blk + 128],
                                ho[:, 2 * k * b + 1 : 2 * (k + 1) * b : 2],
                                start=False,
                                stop=(k == KT - 1),
                                skip_group_check=True,
                            )

                # gate order r, z, hn, in: the joint sigmoid fires right
                # after the z accumulation, in the shadow of hn/in matmuls
                mm_block(w8_sb, w8_base, rz_ps, 0, 0, 4)    # r
                mm_block(w8_sb, w8_base, rz_ps, 4, 16, 4)   # z

                rz_sig = work.tile([128, 8 * b], F32, tag="rz_sig")
                nc.scalar.activation(rz_sig[:], rz_ps[:], sig, scale=1.0 / SCALE)
                rT = rz_sig[:, 0 : 4 * b]
                zT = rz_sig[:, 4 * b : 8 * b]

                # DVE queue order: omz, rhn, pre_n, zh, scan
                nc.vector.tensor_scalar(
                    sd0[:, 1::2], zT, -1.0, 1.0,
                    mybir.AluOpType.mult, mybir.AluOpType.add,
                )

                mm_block(w8_sb, w8_base, hn_ps, 0, 32, 4)   # hn
                rhn = work.tile([128, 4 * b], F32, tag="rhn")
                nc.vector.tensor_mul(rhn[:], rT, hn_ps[:])

                if not first:
                    mm_block(w16_sb, 0, in_ps, 0, 0, 4)     # in
                pre_n = work.tile([128, 4 * b], F32, tag="pre_n")
                nc.vector.tensor_add(pre_n[:], in_ps[:], rhn[:])
                nc.scalar.activation(sd1[:, 0::2], pre_n[:], tanh, scale=1.0 / SCALE)

                nc.vector.tensor_mul(sd1[:, 1::2], zT, ho[:, 1::2])

                ho_new = state.tile([128, 8 * b], F16, tag="ho")
                nc.vector.tensor_tensor_scan(
                    ho_new[:], sd0[:], sd1[:], 0.0,
                    mybir.AluOpType.mult, mybir.AluOpType.add,
                )
                nc.sync.dma_start(out_d[t], ho_new[:])
                ho = ho_new

    if not nc.is_finalized():
        nc.finalize()
    return nc


def _prep_host(encoder_outputs, W_ih, W_hh, b_ih, b_hh, T, n_cores, b):
    """Shard + lay out host inputs; returns per-core in_maps."""
    W_ih = np.asarray(W_ih, dtype=np.float32)
    W_hh = np.asarray(W_hh, dtype=np.float32)
    b_ih = np.asarray(b_ih, dtype=np.float32)
    b_hh = np.asarray(b_hh, dtype=np.float32)
    enc = np.asarray(encoder_outputs, dtype=np.float32)

    import ml_dtypes

    # fp8 gates [r; z; hn] for the loop and [r1; z1; hn1] for step 1, x1024;
    # the in-gate stays fp16 (also x1024 so all PSUM values share the scale)
    W8 = np.concatenate(
        [W_ih[:H] + W_hh[:H], W_ih[H : 2 * H] + W_hh[H : 2 * H], W_hh[2 * H :],
         W_hh[:H], W_hh[H : 2 * H], W_hh[2 * H :]], axis=0,
    )
    Win = W_ih[2 * H :]
    bc_rz = np.concatenate([b_ih[:H] + b_hh[:H], b_ih[H : 2 * H] + b_hh[H : 2 * H]])
    b_hn = b_hh[2 * H :]
    b_in = b_ih[2 * H :]

    def blocks_of(Wm, n_row_tiles, dtype):
        WmT = np.ascontiguousarray(Wm.T * SCALE_HOST)  # (512, rows)
        cols = []
        for tt in range(n_row_tiles):
            for k in range(KT):
                cols.append(WmT[128 * k : 128 * (k + 1), 128 * tt : 128 * (tt + 1)])
        return np.concatenate(cols, axis=1).astype(dtype)

    w8_host = blocks_of(W8, 48, ml_dtypes.float8_e4m3fn)   # (128, 96*128)
    w16_host = blocks_of(Win, 4, np.float16)               # (128, 16*128)

    bst = (np.concatenate([
        bc_rz.reshape(8, 128), b_hn.reshape(4, 128), b_in.reshape(4, 128),
    ], axis=0) * SCALE_HOST).astype(np.float16)  # (16, 128)
    ones8 = np.kron(np.eye(8, dtype=np.float16), np.ones((1, b), np.float16))
    ones4 = np.kron(np.eye(4, dtype=np.float16), np.ones((1, b), np.float16))

    h0 = enc[0, :, -1, :]  # (128, 512)
    in_maps = []
    for c in range(n_cores):
        h0c = h0[c * b : (c + 1) * b]  # (b, 512)
        h0t = np.ascontiguousarray(
            h0c.reshape(b, KT, 128).transpose(2, 1, 0).reshape(128, KT * b)
        ).astype(np.float16)
        h0i = np.zeros((128, 2 * KT * b), np.float16)
        h0i[:, 1::2] = h0t
        in_maps.append({
            "w8": w8_host, "w16": w16_host, "bst": bst,
            "ones8": ones8, "ones4": ones4, "h0t": h0i,
        })
    return in_maps


def _gather(results, T, n_cores, b):
    out = np.empty((T, BATCH, H), dtype=np.float32)
    for c in range(n_cores):
        oc = results[c]["outT"][:, :, 1::2]  # (T, 128, KT*b) fp16, free = [k][j]
        out[:, c * b : (c + 1) * b, :] = (
            oc.astype(np.float32)
            .reshape(T, 128, KT, b).transpose(0, 3, 2, 1).reshape(T, b, H)
        )
    return out


_CACHE = {}


def kernel(encoder_outputs, W_ih, W_hh, b_ih, b_hh, out_len):
    T = int(out_len)
    assert T == T_STEPS, f"built for T={T_STEPS}, got {T}"
    key = (T, N_CORES)
    if key not in _CACHE:
        _CACHE[key] = _build(T, B_LOC)
    nc = _CACHE[key]

    in_maps = _prep_host(encoder_outputs, W_ih, W_hh, b_ih, b_hh,
                         T, N_CORES, B_LOC)
    res = run_bass_kernel_spmd(nc, in_maps, core_ids=list(range(N_CORES)))
    global _LAST_RESULTS
    _LAST_RESULTS = res
    out = _gather(res.results, T, N_CORES, B_LOC)
    return out.reshape(T * BATCH, 1, H)


# revision 7
# speedup vs baseline: 1.2076x; 1.2076x over previous
"""Trainium2 Bass kernel for the GRU decoder (nn_Decoder_13168369730058).

Math (from the reference):
  h0 = encoder_outputs[0, :, -1, :]                       # (128, 512)
  step 1:   h1 = gru_cell(x=0, h0)
  step t>1: h_t = gru_cell(h_{t-1}, h_{t-1})   (carry is (h_new, h_new))

Because x == h from step 2 on, the two GRU matmuls fuse into one:
  g  = h @ Wc.T          Wc = [Wih_r+Whh_r; Wih_z+Whh_z; Whh_n; Wih_n]  (2048, 512)
  r  = sigmoid(g_r + b_r)        b_r = b_ih_r + b_hh_r
  z  = sigmoid(g_z + b_z)
  n  = tanh(g_in + b_in + r * (g_hn + b_hn))     b_in = b_ih_n, b_hn = b_hh_n
  h' = (1 - z) * n + z * h
Step 1 is the same recurrence with Wc -> W_hh and no in-matmul (g_in = 0).

Distribution: data-parallel over batch, 16 rows per core on 8 cores, weights
replicated; the out_len recurrence is local to each core.

On-chip layout is fully transposed (H on partitions, batch on free dim): the
matmul runs with the weight tile as the stationary operand (lhsT = 128x128
fp16 block, FWL) and the transposed hidden state h^T (128, 16) as the moving
operand, producing g^T directly in PSUM.

Step latency = 64 MM-dispatches (~26.6 ns each, dispatch-floor bound) plus
the serial sigmoid/tanh chain, so the chain is squeezed hard:
  - gates issue in r, z, hn, in order; the joint sigmoid(r|z) fires as soon
    as the z accumulation lands, in the shadow of the hn/in matmuls;
  - DVE program order omz, rhn, pre_n, zh keeps the critical ops at the
    queue head the moment their matmul inputs land;
  - the final blend (1-z)*n + z*h is ONE tensor_tensor_scan over
    interleaved operands: out[2j] = n[j], out[2j+1] = omz[j]*n[j] + zh[j],
    with tanh/omz/zh writing directly into the interleaved scan operands;
  - state is fp16 only (no fp32 copy); the output DMA ships the fp16 scan
    tile and the host widens/deinterleaves to fp32;
  - all three gate biases are injected into PSUM by tiny rank-k matmuls so
    there are no DVE bias adds.
"""

import os
import numpy as np

import concourse.bacc as bacc
import concourse.mybir as mybir
import concourse.tile as tile
from concourse.bass_utils import run_bass_kernel_spmd

H = 512
BATCH = 128
N_CORES = int(os.environ.get("GRU_N_CORES", "8"))
T_STEPS = int(os.environ.get("GRU_T_STEPS", "1024"))
B_LOC = BATCH // N_CORES  # local batch per core (16)
SCALE_HOST = 1024.0
KT = H // 128             # 4 k-tiles

F32 = mybir.dt.float32
F16 = mybir.dt.float16
F8 = mybir.dt.float8e4
SCALE = 1024.0  # fp8/fp16 weights & biases stored x1024; descaled in ACT


def _build(T: int, b: int):
    """Build the Bass program: T steps, b batch rows per core."""
    nc = bacc.Bacc()

    w8_d = nc.dram_tensor("w8", [128, 96 * 128], F8, kind="ExternalInput")
    w16_d = nc.dram_tensor("w16", [128, 16 * 128], F16, kind="ExternalInput")
    # bias stationaries: row k of section s = bias[128k:128(k+1)] for that gate
    bst_d = nc.dram_tensor("bst", [16, 128], F16, kind="ExternalInput")
    ones8_d = nc.dram_tensor("ones8", [8, 8 * b], F16, kind="ExternalInput")
    ones4_d = nc.dram_tensor("ones4", [4, 4 * b], F16, kind="ExternalInput")
    h0_d = nc.dram_tensor("h0t", [128, 4 * b], F16, kind="ExternalInput")
    out_d = nc.dram_tensor("outT", [T, 128, 4 * b], F16, kind="ExternalOutput")

    sig = mybir.ActivationFunctionType.Sigmoid
    tanh = mybir.ActivationFunctionType.Tanh

    with tile.TileContext(nc) as tc:
        with (
            tc.tile_pool(name="singles", bufs=1) as singles,
            tc.tile_pool(name="state", bufs=2) as state,
            tc.tile_pool(name="work", bufs=2) as work,
            tc.tile_pool(name="psum", bufs=2, space="PSUM") as psum,
        ):
            w8_sb = singles.tile([128, 96 * 128], F8)
            nc.sync.dma_start(w8_sb[:], w8_d[:])
            w16_sb = singles.tile([128, 16 * 128], F16)
            nc.sync.dma_start(w16_sb[:], w16_d[:])
            brz_sb = singles.tile([8, 128], F16)
            nc.sync.dma_start(brz_sb[:], bst_d[0:8])
            bhn_sb = singles.tile([4, 128], F16)
            nc.sync.dma_start(bhn_sb[:], bst_d[8:12])
            bin_sb = singles.tile([4, 128], F16)
            nc.sync.dma_start(bin_sb[:], bst_d[12:16])
            ones8_sb = singles.tile([8, 8 * b], F16)
            nc.sync.dma_start(ones8_sb[:], ones8_d[:])
            ones4_sb = singles.tile([4, 4 * b], F16)
            nc.sync.dma_start(ones4_sb[:], ones4_d[:])

            ho = state.tile([128, 4 * b], F16, tag="ho")
            nc.sync.dma_start(ho[:], h0_d[:])

            # Warm-up: hardware allows ONE embedded sync wait per instruction;
            # have each engine observe the init DMA queues here so loop
            # instructions carry a single cross-engine wait.
            warm_ps = psum.tile([128, 8], F32, tag="warm", bufs=1)
            nc.tensor.matmul(warm_ps[:, 0:8], w8_sb[:, 0:128], ho[:, 0:8],
                             start=True, stop=True)
            nc.tensor.matmul(warm_ps[:, 0:8], w16_sb[:, 0:128], w16_sb[:, 0:8],
                             start=True, stop=True)
            nc.tensor.matmul(warm_ps[:, 0:1], brz_sb[:, 0:128], ones8_sb[:, 0:1],
                             start=True, stop=True)
            nc.tensor.matmul(warm_ps[:, 0:1], bhn_sb[:, 0:128], ones4_sb[:, 0:1],
                             start=True, stop=True)
            nc.tensor.matmul(warm_ps[:, 0:1], bin_sb[:, 0:128], ones4_sb[:, 0:1],
                             start=True, stop=True)

            for t in range(T):
                first = t == 0
                w8_base = (48 * 128) if first else 0

                rz_ps = psum.tile([128, 8 * b], F32, tag="rz")
                hn_ps = psum.tile([128, 4 * b], F32, tag="hn")
                in_ps = psum.tile([128, 4 * b], F32, tag="in")

                # bias seeds (start=True writes bias, sets has_written)
                nc.tensor.matmul(rz_ps[:], brz_sb[:], ones8_sb[:],
                                 start=True, stop=False, skip_group_check=True)
                nc.tensor.matmul(hn_ps[:], bhn_sb[:], ones4_sb[:],
                                 start=True, stop=False, skip_group_check=True)
                nc.tensor.matmul(in_ps[:], bin_sb[:], ones4_sb[:],
                                 start=True, stop=first, skip_group_check=True)

                def mm_block(w_sb, base, ps, ps_off, blk0, ntiles):
                    for tt in range(ntiles):
                        for k in range(KT):
                            blk = base + (blk0 + tt * KT + k) * 128
                            nc.tensor.matmul(
                                ps[:, (ps_off + tt) * b : (ps_off + tt + 1) * b],
                                w_sb[:, blk : blk + 128],
                                ho[:, k * b : (k + 1) * b],
                                start=False,
                                stop=(k == KT - 1),
                                skip_group_check=True,
                            )

                # gate order r, z, hn, in: the joint sigmoid fires right
                # after the z accumulation, in the shadow of hn/in matmuls
                mm_block(w8_sb, w8_base, rz_ps, 0, 0, 4)    # r
                mm_block(w8_sb, w8_base, rz_ps, 4, 16, 4)   # z

                rz_sig = work.tile([128, 8 * b], F32, tag="rz_sig")
                nc.scalar.activation(rz_sig[:], rz_ps[:], sig, scale=1.0 / SCALE)
                rT = rz_sig[:, 0 : 4 * b]
                zT = rz_sig[:, 4 * b : 8 * b]

                # DVE queue order: omz, rhn, pre_n, zh, on, h'
                omz = work.tile([128, 4 * b], F16, tag="omz")
                nc.vector.tensor_scalar(
                    omz[:], zT, -1.0, 1.0,
                    mybir.AluOpType.mult, mybir.AluOpType.add,
                )

                mm_block(w8_sb, w8_base, hn_ps, 0, 32, 4)   # hn
                rhn = work.tile([128, 4 * b], F32, tag="rhn")
                nc.vector.tensor_mul(rhn[:], rT, hn_ps[:])

                if not first:
                    mm_block(w16_sb, 0, in_ps, 0, 0, 4)     # in
                pre_n = work.tile([128, 4 * b], F32, tag="pre_n")
                nc.vector.tensor_add(pre_n[:], in_ps[:], rhn[:])
                n_t = work.tile([128, 4 * b], F16, tag="n")
                nc.scalar.activation(n_t[:], pre_n[:], tanh, scale=1.0 / SCALE)

                zh = work.tile([128, 4 * b], F16, tag="zh")
                nc.vector.tensor_mul(zh[:], zT, ho[:])

                on = work.tile([128, 4 * b], F16, tag="on")
                nc.vector.tensor_mul(on[:], omz[:], n_t[:])
                ho_new = state.tile([128, 4 * b], F16, tag="ho")
                nc.vector.tensor_add(ho_new[:], on[:], zh[:])
                nc.sync.dma_start(out_d[t], ho_new[:])
                ho = ho_new

    if not nc.is_finalized():
        nc.finalize()
    return nc


def _prep_host(encoder_outputs, W_ih, W_hh, b_ih, b_hh, T, n_cores, b):
    """Shard + lay out host inputs; returns per-core in_maps."""
    W_ih = np.asarray(W_ih, dtype=np.float32)
    W_hh = np.asarray(W_hh, dtype=np.float32)
    b_ih = np.asarray(b_ih, dtype=np.float32)
    b_hh = np.asarray(b_hh, dtype=np.float32)
    enc = np.asarray(encoder_outputs, dtype=np.float32)

    import ml_dtypes

    # fp8 gates [r; z; hn] for the loop and [r1; z1; hn1] for step 1, x1024;
    # the in-gate stays fp16 (also x1024 so all PSUM values share the scale)
    W8 = np.concatenate(
        [W_ih[:H] + W_hh[:H], W_ih[H : 2 * H] + W_hh[H : 2 * H], W_hh[2 * H :],
         W_hh[:H], W_hh[H : 2 * H], W_hh[2 * H :]], axis=0,
    )
    Win = W_ih[2 * H :]
    bc_rz = np.concatenate([b_ih[:H] + b_hh[:H], b_ih[H : 2 * H] + b_hh[H : 2 * H]])
    b_hn = b_hh[2 * H :]
    b_in = b_ih[2 * H :]

    def blocks_of(Wm, n_row_tiles, dtype):
        WmT = np.ascontiguousarray(Wm.T * SCALE_HOST)  # (512, rows)
        cols = []
        for tt in range(n_row_tiles):
            for k in range(KT):
                cols.append(WmT[128 * k : 128 * (k + 1), 128 * tt : 128 * (tt + 1)])
        return np.concatenate(cols, axis=1).astype(dtype)

    w8_host = blocks_of(W8, 48, ml_dtypes.float8_e4m3fn)   # (128, 96*128)
    w16_host = blocks_of(Win, 4, np.float16)               # (128, 16*128)

    bst = (np.concatenate([
        bc_rz.reshape(8, 128), b_hn.reshape(4, 128), b_in.reshape(4, 128),
    ], axis=0) * SCALE_HOST).astype(np.float16)  # (16, 128)
    ones8 = np.kron(np.eye(8, dtype=np.float16), np.ones((1, b), np.float16))
    ones4 = np.kron(np.eye(4, dtype=np.float16), np.ones((1, b), np.float16))

    h0 = enc[0, :, -1, :]  # (128, 512)
    in_maps = []
    for c in range(n_cores):
        h0c = h0[c * b : (c + 1) * b]  # (b, 512)
        h0t = np.ascontiguousarray(
            h0c.reshape(b, KT, 128).transpose(2, 1, 0).reshape(128, KT * b)
        ).astype(np.float16)
        in_maps.append({
            "w8": w8_host, "w16": w16_host, "bst": bst,
            "ones8": ones8, "ones4": ones4, "h0t": h0t,
        })
    return in_maps


def _gather(results, T, n_cores, b):
    out = np.empty((T, BATCH, H), dtype=np.float32)
    for c in range(n_cores):
        oc = results[c]["outT"]  # (T, 128, KT*b) fp16, free = [k][j]
        out[:, c * b : (c + 1) * b, :] = (
            oc.astype(np.float32)
            .reshape(T, 128, KT, b).transpose(0, 3, 2, 1).reshape(T, b, H)
        )
    return out


_CACHE = {}


def kernel(encoder_outputs, W_ih, W_hh, b_ih, b_hh, out_len):
    T = int(out_len)
    assert T == T_STEPS, f"built for T={T_STEPS}, got {T}"
    key = (T, N_CORES)
    if key not in _CACHE:
        _CACHE[key] = _build(T, B_LOC)
    nc = _CACHE[key]

    in_maps = _prep_host(encoder_outputs, W_ih, W_hh, b_ih, b_hh,
                         T, N_CORES, B_LOC)
    res = run_bass_kernel_spmd(nc, in_maps, core_ids=list(range(N_CORES)))
    global _LAST_RESULTS
    _LAST_RESULTS = res
    out = _gather(res.results, T, N_CORES, B_LOC)
    return out.reshape(T * BATCH, 1, H)
